# revision 4
# baseline (speedup 1.0000x reference)
"""Affinity-propagate (SPN) Trainium2 Bass kernel.

Computation (per batch element, see reference):
    w = g / conv3x3_ones(|g|)          # gates, [8, H, W], computed once
    d_{k+1} = max_c conv3x3_ones(w_c * d_k)   # 8 iterations

Distribution: pure data parallel, batch element b -> NeuronCore b (8 cores).

Per-core mapping:
  - H=352 rows live on SBUF partitions as 3 overlapping tiles
    (rows 0..127, 126..253, 252..351).  The 3x3 conv's H-direction sum is a
    tri-band matrix matmul on the tensor engine (contraction over the
    partition/H axis); output rows at tile seams that lack a cross-tile
    neighbour are invalid and are instead produced by the adjacent tile, with
    4 one-row SBUF->SBUF DMA "seam" copies per iteration.
  - The W-direction sum is folded into the same matmuls: 3 PSUM-accumulating
    matmuls with the moving operand shifted by -1/0/+1 columns (W is padded
    by one zero column on each side).
  - Work split: DVE computes p = w*d (fp32 in, float32r out) and the
    channel-max tree; PE does all conv sums (float32r, 1 cycle/col);
    ScalarE evacuates PSUM->SBUF; DMA does seam rows.
  - d is stored fp32 and double-buffered across iterations; only p is
    rounded to float32r (~1e-4 relative per conv), keeping the final
    relative error ~1e-4..1e-3.
"""
from contextlib import ExitStack

import numpy as np

import concourse.bacc as bacc
import concourse.mybir as mybir
import concourse.tile as tile
from concourse.bass_utils import run_bass_kernel_spmd

F32 = mybir.dt.float32
F32R = mybir.dt.float32r

B, C, H, W = 8, 8, 352, 1216
WB = W + 2  # zero-padded width
N_ITERS = 8
N_CORES = 8

ROW_BASE = [0, 126, 252]       # first global row of each H tile
ROWS = [128, 128, 100]         # partitions used by each H tile
MV = [(0, 127), (1, 127), (1, 100)]   # valid conv-output partition range
CHUNKS = [(0, 512), (512, 448), (960, 256)]  # (start col, width); all >=256 for f32r speed


def _build_nc():
    nc = bacc.Bacc("TRN2", target_bir_lowering=False, debug=False,
                   num_devices=N_CORES)
    g = nc.dram_tensor("g", [C, H, W], F32, kind="ExternalInput").ap()
    d_in = nc.dram_tensor("d", [H, W], F32, kind="ExternalInput").ap()
    band = nc.dram_tensor("band", [128, 128], F32R, kind="ExternalInput").ap()
    out = nc.dram_tensor("out", [H, W], F32, kind="ExternalOutput").ap()

    with tile.TileContext(nc) as tc, ExitStack() as ctx:
        pw = ctx.enter_context(tc.tile_pool(name="w", bufs=1))
        pd = ctx.enter_context(tc.tile_pool(name="d", bufs=1))
        pc = ctx.enter_context(tc.tile_pool(name="const", bufs=1))
        pp = ctx.enter_context(tc.tile_pool(name="p", bufs=4))
        pprop = ctx.enter_context(tc.tile_pool(name="prop", bufs=2))
        ptree1 = ctx.enter_context(tc.tile_pool(name="tree1", bufs=1))
        ptree2 = ctx.enter_context(tc.tile_pool(name="tree2", bufs=2))
        psum = ctx.enter_context(tc.tile_pool(name="psum", bufs=8, space="PSUM"))

        A = pc.tile([128, 128], F32R, tag="band", name="bandt")
        nc.sync.dma_start(A[:], band[:])

        wt = [pw.tile([128, C, WB], F32, tag=f"w{t}", name=f"w{t}") for t in range(3)]
        # double-buffered depth: dts[i][t], padded width
        dts = [[pd.tile([128, WB], F32, tag=f"d{i}{t}", name=f"d{i}{t}") for t in range(3)]
               for i in range(2)]

        # ---- zero pad columns (and load inputs) ----
        for t in range(3):
            R, rb = ROWS[t], ROW_BASE[t]
            # pad cols 0 and WB-1 of w and both d buffers must be zero
            nc.vector.memset(wt[t][:, :, 0:1], 0.0)
            nc.vector.memset(wt[t][:, :, WB - 1:WB], 0.0)
            for i in range(2):
                nc.vector.memset(dts[i][t][:, 0:1], 0.0)
                nc.vector.memset(dts[i][t][:, WB - 1:WB], 0.0)
            nc.sync.dma_start(
                wt[t][0:R, :, 1:W + 1],
                g[:, rb:rb + R, :].rearrange("c r w -> r c w"))
            nc.sync.dma_start(dts[0][t][0:R, 1:W + 1], d_in[rb:rb + R, :])

        # ---- phase 0: w = g / conv3x3_ones(|g|) ----
        for t in range(3):
            R = ROWS[t]
            for c in range(C):
                s_plane = pprop.tile([128, W], F32, tag="prop", name="splane")
                for (J, N) in CHUNKS:
                    p = pp.tile([128, 514], F32R, tag="p", name="p")
                    nc.scalar.activation(
                        p[0:R, 0:N + 2], wt[t][0:R, c, J:J + N + 2],
                        mybir.ActivationFunctionType.Abs)
                    ps = psum.tile([128, N], F32, tag="ps", name="ps")
                    for s in range(3):
                        nc.tensor.matmul(ps[0:R, 0:N], A[0:R, 0:R],
                                         p[0:R, s:s + N],
                                         start=(s == 0), stop=(s == 2))
                    nc.scalar.copy(s_plane[0:R, J:J + N], ps[0:R, 0:N])
                rcp = pprop.tile([128, W], F32, tag="prop", name="rcp")
                nc.vector.reciprocal_approx_fast(out=rcp[0:R, :],
                                                 in_=s_plane[0:R, :])
                nc.vector.tensor_mul(wt[t][0:R, c, 1:W + 1],
                                     wt[t][0:R, c, 1:W + 1], rcp[0:R, :])
        # w seam rows (rows whose conv window crossed a tile boundary)
        nc.sync.dma_start(wt[0][127:128, :, 1:W + 1], wt[1][1:2, :, 1:W + 1])
        nc.sync.dma_start(wt[1][0:1, :, 1:W + 1], wt[0][126:127, :, 1:W + 1])
        nc.sync.dma_start(wt[1][127:128, :, 1:W + 1], wt[2][1:2, :, 1:W + 1])
        nc.sync.dma_start(wt[2][0:1, :, 1:W + 1], wt[1][126:127, :, 1:W + 1])

        # ---- phase 1: 8 propagation iterations ----
        for k in range(N_ITERS):
            dsrc = dts[k % 2]
            ddst = dts[(k + 1) % 2]
            for t in range(3):
                R = ROWS[t]
                for (J, N) in CHUNKS:
                    prop = pprop.tile([128, C, 512], F32, tag="prop", name="prop")
                    for c in range(C):
                        p = pp.tile([128, 514], F32R, tag="p", name="p")
                        nc.vector.tensor_mul(p[0:R, 0:N + 2],
                                             wt[t][0:R, c, J:J + N + 2],
                                             dsrc[t][0:R, J:J + N + 2])
                        ps = psum.tile([128, N], F32, tag="ps", name="ps")
                        for s in range(3):
                            nc.tensor.matmul(ps[0:R, 0:N], A[0:R, 0:R],
                                             p[0:R, s:s + N],
                                             start=(s == 0), stop=(s == 2))
                        nc.scalar.copy(prop[0:R, c, 0:N], ps[0:R, 0:N])
                    t1 = ptree1.tile([128, 4, 512], F32, tag="t1", name="t1")
                    nc.vector.tensor_max(t1[0:R, :, 0:N],
                                         prop[0:R, 0:4, 0:N],
                                         prop[0:R, 4:8, 0:N])
                    t2 = ptree2.tile([128, 2, 512], F32, tag="t2", name="t2")
                    nc.vector.tensor_max(t2[0:R, :, 0:N],
                                         t1[0:R, 0:2, 0:N],
                                         t1[0:R, 2:4, 0:N])
                    # full partition range (base must be 32-aligned);
                    # junk seam rows are overwritten by the seam DMAs below
                    nc.vector.tensor_max(ddst[t][0:R, 1 + J:1 + J + N],
                                         t2[0:R, 0, 0:N],
                                         t2[0:R, 1, 0:N])
            # seam rows of the freshly written buffer
            nc.sync.dma_start(ddst[0][127:128, 1:W + 1], ddst[1][1:2, 1:W + 1])
            nc.sync.dma_start(ddst[1][0:1, 1:W + 1], ddst[0][126:127, 1:W + 1])
            nc.sync.dma_start(ddst[1][127:128, 1:W + 1], ddst[2][1:2, 1:W + 1])
            nc.sync.dma_start(ddst[2][0:1, 1:W + 1], ddst[1][126:127, 1:W + 1])

        dfin = dts[N_ITERS % 2]
        nc.sync.dma_start(out[0:128, :], dfin[0][0:128, 1:W + 1])
        nc.sync.dma_start(out[128:254, :], dfin[1][2:128, 1:W + 1])
        nc.sync.dma_start(out[254:352, :], dfin[2][2:100, 1:W + 1])

    nc.compile()
    return nc


def _band_matrix():
    a = np.zeros((128, 128), dtype=np.float32)
    idx = np.arange(128)
    a[idx, idx] = 1.0
    a[idx[:-1], idx[:-1] + 1] = 1.0
    a[idx[1:], idx[1:] - 1] = 1.0
    return a


_NC_CACHE = None


def kernel(guidance: np.ndarray, blur_depth: np.ndarray) -> np.ndarray:
    """Full inputs in, full output out. Shards batch across 8 NeuronCores."""
    global _NC_CACHE
    assert guidance.shape == (B, C, H, W), guidance.shape
    assert blur_depth.shape == (B, 1, H, W), blur_depth.shape
    if _NC_CACHE is None:
        _NC_CACHE = _build_nc()
    nc = _NC_CACHE
    band = _band_matrix()
    in_maps = [
        {
            "g": np.ascontiguousarray(guidance[b], dtype=np.float32),
            "d": np.ascontiguousarray(blur_depth[b, 0], dtype=np.float32),
            "band": band,
        }
        for b in range(B)
    ]
    res = run_bass_kernel_spmd(nc, in_maps, core_ids=list(range(N_CORES)))
    out = np.stack([res.results[b]["out"] for b in range(B)])[:, None]
    return out.astype(np.float32)


# revision 6
# speedup vs baseline: 1.1629x; 1.1629x over previous
"""Affinity-propagate (SPN) Trainium2 Bass kernel.

Computation (per batch element, see reference):
    w = g / conv3x3_ones(|g|)          # gates, [8, H, W], computed once
    d_{k+1} = max_c conv3x3_ones(w_c * d_k)   # 8 iterations

Distribution: pure data parallel, batch element b -> NeuronCore b (8 cores).

Per-core mapping:
  - H=352 rows live on SBUF partitions as 3 overlapping tiles
    (rows 0..127, 126..253, 252..351).  The 3x3 conv's H-direction sum is a
    tri-band matrix matmul on the tensor engine (contraction over the
    partition/H axis); output rows at tile seams that lack a cross-tile
    neighbour are invalid and are instead produced by the adjacent tile, with
    4 one-row SBUF->SBUF DMA "seam" copies per iteration.
  - The W-direction sum is folded into the same matmuls: 3 PSUM-accumulating
    matmuls with the moving operand shifted by -1/0/+1 columns (W is padded
    by one zero column on each side).
  - Work split: DVE computes p = w*d (fp32 in, float32r out, one op per
    (tile, channel) plane) and the channel-max tree; PE does all conv sums
    (float32r, 1 cycle/col at N>=256); ScalarE evacuates PSUM->SBUF;
    DMA does seam rows.
  - d is stored fp32, updated in place (trace order makes WAR/RAW safe);
    only p is rounded to float32r (~1e-4 relative per conv), keeping the
    final relative error ~3e-4.
"""
from contextlib import ExitStack

import numpy as np

import concourse.bacc as bacc
import concourse.mybir as mybir
import concourse.tile as tile
from concourse.bass_utils import run_bass_kernel_spmd

F32 = mybir.dt.float32
F32R = mybir.dt.float32r

B, C, H, W = 8, 8, 352, 1216
WB = W + 2  # zero-padded width
N_ITERS = 8
N_CORES = 8

ROW_BASE = [0, 126, 252]       # first global row of each H tile
ROWS = [128, 128, 100]         # partitions used by each H tile
CHUNKS = [(0, 512), (512, 448), (960, 256)]  # (start col, width); >=256 for f32r speed


def _build_nc():
    nc = bacc.Bacc("TRN2", target_bir_lowering=False, debug=False,
                   num_devices=N_CORES)
    g = nc.dram_tensor("g", [C, H, W], F32, kind="ExternalInput").ap()
    d_in = nc.dram_tensor("d", [H, W], F32, kind="ExternalInput").ap()
    band = nc.dram_tensor("band", [128, 128], F32R, kind="ExternalInput").ap()
    out = nc.dram_tensor("out", [H, W], F32, kind="ExternalOutput").ap()

    with tile.TileContext(nc) as tc, ExitStack() as ctx:
        pw = ctx.enter_context(tc.tile_pool(name="w", bufs=1))
        pd = ctx.enter_context(tc.tile_pool(name="d", bufs=1))
        pc = ctx.enter_context(tc.tile_pool(name="const", bufs=1))
        pp = ctx.enter_context(tc.tile_pool(name="p", bufs=3))
        pprop = ctx.enter_context(tc.tile_pool(name="prop", bufs=1))
        ptree1 = ctx.enter_context(tc.tile_pool(name="tree1", bufs=2))
        ptree2 = ctx.enter_context(tc.tile_pool(name="tree2", bufs=1))
        psum = ctx.enter_context(tc.tile_pool(name="psum", bufs=6, space="PSUM"))

        A = pc.tile([128, 128], F32R, tag="band", name="bandt")
        nc.sync.dma_start(A[:], band[:])

        wt = [pw.tile([128, C, WB], F32, tag=f"w{t}", name=f"w{t}")
              for t in range(3)]
        dt_ = [pd.tile([128, WB], F32, tag=f"d{t}", name=f"d{t}")
               for t in range(3)]

        # ---- zero pad columns, load inputs ----
        for t in range(3):
            R, rb = ROWS[t], ROW_BASE[t]
            nc.vector.memset(wt[t][:, :, 0:1], 0.0)
            nc.vector.memset(wt[t][:, :, WB - 1:WB], 0.0)
            nc.vector.memset(dt_[t][:, 0:1], 0.0)
            nc.vector.memset(dt_[t][:, WB - 1:WB], 0.0)
            nc.sync.dma_start(
                wt[t][0:R, :, 1:W + 1],
                g[:, rb:rb + R, :].rearrange("c r w -> r c w"))
            nc.sync.dma_start(dt_[t][0:R, 1:W + 1], d_in[rb:rb + R, :])

        # ---- phase 0: w = g / conv3x3_ones(|g|) ----
        for t in range(3):
            R = ROWS[t]
            for c in range(C):
                p = pp.tile([128, WB], F32R, tag="p", name="p")
                nc.scalar.activation(p[0:R, :], wt[t][0:R, c, :],
                                     mybir.ActivationFunctionType.Abs)
                s_buf = ptree1.tile([128, W], F32, tag="t1", name="sbuf_")
                for (J, N) in CHUNKS:
                    ps = psum.tile([128, 512], F32, tag="ps", name="ps")
                    for s in range(3):
                        nc.tensor.matmul(ps[0:R, 0:N], A[0:R, 0:R],
                                         p[0:R, J + s:J + s + N],
                                         start=(s == 0), stop=(s == 2))
                    nc.scalar.copy(s_buf[0:R, J:J + N], ps[0:R, 0:N])
                rcp = ptree1.tile([128, W], F32, tag="t1", name="rcp")
                nc.vector.reciprocal_approx_fast(out=rcp[0:R, :],
                                                 in_=s_buf[0:R, :])
                nc.vector.tensor_mul(wt[t][0:R, c, 1:W + 1],
                                     wt[t][0:R, c, 1:W + 1], rcp[0:R, :])
        # w seam rows
        nc.sync.dma_start(wt[0][127:128, :, 1:W + 1], wt[1][1:2, :, 1:W + 1])
        nc.sync.dma_start(wt[1][0:1, :, 1:W + 1], wt[0][126:127, :, 1:W + 1])
        nc.sync.dma_start(wt[1][127:128, :, 1:W + 1], wt[2][1:2, :, 1:W + 1])
        nc.sync.dma_start(wt[2][0:1, :, 1:W + 1], wt[1][126:127, :, 1:W + 1])

        # ---- phase 1: 8 propagation iterations ----
        for k in range(N_ITERS):
            for t in range(3):
                R = ROWS[t]
                prop = pprop.tile([128, C, W], F32, tag="prop", name="prop")
                for c in range(C):
                    p = pp.tile([128, WB], F32R, tag="p", name="p")
                    nc.vector.tensor_mul(p[0:R, :], wt[t][0:R, c, :],
                                         dt_[t][0:R, :])
                    for (J, N) in CHUNKS:
                        ps = psum.tile([128, 512], F32, tag="ps", name="ps")
                        for s in range(3):
                            nc.tensor.matmul(ps[0:R, 0:N], A[0:R, 0:R],
                                             p[0:R, J + s:J + s + N],
                                             start=(s == 0), stop=(s == 2))
                        nc.scalar.copy(prop[0:R, c, J:J + N], ps[0:R, 0:N])
                for (J, N) in CHUNKS:
                    t1 = ptree1.tile([128, 4, 512], F32, tag="t1", name="t1")
                    nc.vector.tensor_max(t1[0:R, :, 0:N],
                                         prop[0:R, 0:4, J:J + N],
                                         prop[0:R, 4:8, J:J + N])
                    t2 = ptree2.tile([128, 2, 512], F32, tag="t2", name="t2")
                    nc.vector.tensor_max(t2[0:R, :, 0:N],
                                         t1[0:R, 0:2, 0:N],
                                         t1[0:R, 2:4, 0:N])
                    # junk seam rows are fixed by the seam DMAs below
                    nc.vector.tensor_max(dt_[t][0:R, 1 + J:1 + J + N],
                                         t2[0:R, 0, 0:N],
                                         t2[0:R, 1, 0:N])
            # seam rows
            nc.sync.dma_start(dt_[0][127:128, 1:W + 1], dt_[1][1:2, 1:W + 1])
            nc.sync.dma_start(dt_[1][0:1, 1:W + 1], dt_[0][126:127, 1:W + 1])
            nc.sync.dma_start(dt_[1][127:128, 1:W + 1], dt_[2][1:2, 1:W + 1])
            nc.sync.dma_start(dt_[2][0:1, 1:W + 1], dt_[1][126:127, 1:W + 1])

        nc.sync.dma_start(out[0:128, :], dt_[0][0:128, 1:W + 1])
        nc.sync.dma_start(out[128:254, :], dt_[1][2:128, 1:W + 1])
        nc.sync.dma_start(out[254:352, :], dt_[2][2:100, 1:W + 1])

    nc.compile()
    return nc


def _band_matrix():
    a = np.zeros((128, 128), dtype=np.float32)
    idx = np.arange(128)
    a[idx, idx] = 1.0
    a[idx[:-1], idx[:-1] + 1] = 1.0
    a[idx[1:], idx[1:] - 1] = 1.0
    return a


_NC_CACHE = None


def kernel(guidance: np.ndarray, blur_depth: np.ndarray) -> np.ndarray:
    """Full inputs in, full output out. Shards batch across 8 NeuronCores."""
    global _NC_CACHE
    assert guidance.shape == (B, C, H, W), guidance.shape
    assert blur_depth.shape == (B, 1, H, W), blur_depth.shape
    if _NC_CACHE is None:
        _NC_CACHE = _build_nc()
    nc = _NC_CACHE
    band = _band_matrix()
    in_maps = [
        {
            "g": np.ascontiguousarray(guidance[b], dtype=np.float32),
            "d": np.ascontiguousarray(blur_depth[b, 0], dtype=np.float32),
            "band": band,
        }
        for b in range(B)
    ]
    res = run_bass_kernel_spmd(nc, in_maps, core_ids=list(range(N_CORES)))
    out = np.stack([res.results[b]["out"] for b in range(B)])[:, None]
    return out.astype(np.float32)


# revision 7
# speedup vs baseline: 1.2165x; 1.0460x over previous
"""Affinity-propagate (SPN) Trainium2 Bass kernel.

Computation (per batch element, see reference):
    w = g / conv3x3_ones(|g|)          # gates, [8, H, W], computed once
    d_{k+1} = max_c conv3x3_ones(w_c * d_k)   # 8 iterations

Distribution: pure data parallel, batch element b -> NeuronCore b (8 cores).

Per-core mapping:
  - H=352 rows live on SBUF partitions as 3 overlapping tiles
    (rows 0..127, 126..253, 252..351).  The 3x3 conv's H-direction sum is a
    tri-band matrix matmul on the tensor engine (contraction over the
    partition/H axis); output rows at tile seams that lack a cross-tile
    neighbour are invalid and are instead produced by the adjacent tile, with
    4 one-row SBUF->SBUF DMA "seam" copies per iteration.
  - The W-direction sum is folded into the same matmuls: 3 PSUM-accumulating
    matmuls with the moving operand shifted by -1/0/+1 columns (W is padded
    by one zero column on each side).
  - Work split: DVE computes p = w*d (fp32 in, float32r out, one op per
    (tile, channel) plane) and the channel-max tree; PE does all conv sums
    (float32r, 1 cycle/col at N>=256); ScalarE evacuates PSUM->SBUF;
    DMA does seam rows.
  - d is stored fp32, updated in place (trace order makes WAR/RAW safe);
    only p is rounded to float32r (~1e-4 relative per conv), keeping the
    final relative error ~3e-4.
"""
from contextlib import ExitStack

import numpy as np

import concourse.bacc as bacc
import concourse.mybir as mybir
import concourse.tile as tile
from concourse.bass_utils import run_bass_kernel_spmd

F32 = mybir.dt.float32
F32R = mybir.dt.float32r

B, C, H, W = 8, 8, 352, 1216
WB = W + 2  # zero-padded width
N_ITERS = 8
N_CORES = 8

ROW_BASE = [0, 126, 252]       # first global row of each H tile
ROWS = [128, 128, 100]         # partitions used by each H tile
CHUNKS = [(0, 512), (512, 448), (960, 256)]  # (start col, width); >=256 for f32r speed


def _build_nc():
    nc = bacc.Bacc("TRN2", target_bir_lowering=False, debug=False,
                   num_devices=N_CORES)
    g = nc.dram_tensor("g", [C, H, W], F32, kind="ExternalInput").ap()
    d_in = nc.dram_tensor("d", [H, W], F32, kind="ExternalInput").ap()
    band = nc.dram_tensor("band", [128, 128], F32R, kind="ExternalInput").ap()
    out = nc.dram_tensor("out", [H, W], F32, kind="ExternalOutput").ap()

    with tile.TileContext(nc) as tc, ExitStack() as ctx:
        pw = ctx.enter_context(tc.tile_pool(name="w", bufs=1))
        pd = ctx.enter_context(tc.tile_pool(name="d", bufs=1))
        pc = ctx.enter_context(tc.tile_pool(name="const", bufs=1))
        pp = ctx.enter_context(tc.tile_pool(name="p", bufs=4))
        pprop = ctx.enter_context(tc.tile_pool(name="prop", bufs=4))
        ptree1 = ctx.enter_context(tc.tile_pool(name="tree1", bufs=2))
        prm = ctx.enter_context(tc.tile_pool(name="rm", bufs=2))
        psum = ctx.enter_context(tc.tile_pool(name="psum", bufs=8, space="PSUM"))

        A = pc.tile([128, 128], F32R, tag="band", name="bandt")
        nc.sync.dma_start(A[:], band[:])

        wt = [pw.tile([128, C, WB], F32, tag=f"w{t}", name=f"w{t}")
              for t in range(3)]
        dt_ = [pd.tile([128, WB], F32, tag=f"d{t}", name=f"d{t}")
               for t in range(3)]

        # ---- zero pad columns, load inputs ----
        for t in range(3):
            R, rb = ROWS[t], ROW_BASE[t]
            nc.vector.memset(wt[t][:, :, 0:1], 0.0)
            nc.vector.memset(wt[t][:, :, WB - 1:WB], 0.0)
            nc.vector.memset(dt_[t][:, 0:1], 0.0)
            nc.vector.memset(dt_[t][:, WB - 1:WB], 0.0)
            nc.sync.dma_start(
                wt[t][0:R, :, 1:W + 1],
                g[:, rb:rb + R, :].rearrange("c r w -> r c w"))
            nc.sync.dma_start(dt_[t][0:R, 1:W + 1], d_in[rb:rb + R, :])

        # ---- phase 0: w = g / conv3x3_ones(|g|) ----
        for t in range(3):
            R = ROWS[t]
            for c in range(C):
                p = pp.tile([128, WB], F32R, tag="p", name="p")
                nc.scalar.activation(p[0:R, :], wt[t][0:R, c, :],
                                     mybir.ActivationFunctionType.Abs)
                s_buf = ptree1.tile([128, W], F32, tag="t1", name="sbuf_")
                for (J, N) in CHUNKS:
                    ps = psum.tile([128, 512], F32, tag="ps", name="ps")
                    for s in range(3):
                        nc.tensor.matmul(ps[0:R, 0:N], A[0:R, 0:R],
                                         p[0:R, J + s:J + s + N],
                                         start=(s == 0), stop=(s == 2))
                    nc.scalar.copy(s_buf[0:R, J:J + N], ps[0:R, 0:N])
                rcp = ptree1.tile([128, W], F32, tag="t1", name="rcp")
                nc.vector.reciprocal_approx_fast(out=rcp[0:R, :],
                                                 in_=s_buf[0:R, :])
                nc.vector.tensor_mul(wt[t][0:R, c, 1:W + 1],
                                     wt[t][0:R, c, 1:W + 1], rcp[0:R, :])
        # w seam rows
        nc.sync.dma_start(wt[0][127:128, :, 1:W + 1], wt[1][1:2, :, 1:W + 1])
        nc.sync.dma_start(wt[1][0:1, :, 1:W + 1], wt[0][126:127, :, 1:W + 1])
        nc.sync.dma_start(wt[1][127:128, :, 1:W + 1], wt[2][1:2, :, 1:W + 1])
        nc.sync.dma_start(wt[2][0:1, :, 1:W + 1], wt[1][126:127, :, 1:W + 1])

        # ---- phase 1: 8 propagation iterations ----
        for k in range(N_ITERS):
            for t in range(3):
                R = ROWS[t]
                props = []
                for c in range(C):
                    p = pp.tile([128, WB], F32R, tag="p", name="p")
                    nc.vector.tensor_mul(p[0:R, :], wt[t][0:R, c, :],
                                         dt_[t][0:R, :])
                    prop = pprop.tile([128, W], F32, tag="prop", name="prop")
                    for (J, N) in CHUNKS:
                        ps = psum.tile([128, 512], F32, tag="ps", name="ps")
                        for s in range(3):
                            nc.tensor.matmul(ps[0:R, 0:N], A[0:R, 0:R],
                                             p[0:R, J + s:J + s + N],
                                             start=(s == 0), stop=(s == 2))
                        nc.scalar.copy(prop[0:R, J:J + N], ps[0:R, 0:N])
                    props.append(prop)
                    # incremental channel max; last step writes d in place
                    # (junk seam rows fixed by the seam DMAs below)
                    if c == 1:
                        rm = prm.tile([128, W], F32, tag="rm", name="rm")
                        nc.vector.tensor_max(rm[0:R, :], props[0][0:R, :],
                                             props[1][0:R, :])
                    elif c in (2, 3, 4, 5, 6):
                        nc.vector.tensor_max(rm[0:R, :], rm[0:R, :],
                                             props[c][0:R, :])
                    elif c == 7:
                        nc.vector.tensor_max(dt_[t][0:R, 1:W + 1],
                                             rm[0:R, :], props[7][0:R, :])
            # seam rows
            nc.sync.dma_start(dt_[0][127:128, 1:W + 1], dt_[1][1:2, 1:W + 1])
            nc.sync.dma_start(dt_[1][0:1, 1:W + 1], dt_[0][126:127, 1:W + 1])
            nc.sync.dma_start(dt_[1][127:128, 1:W + 1], dt_[2][1:2, 1:W + 1])
            nc.sync.dma_start(dt_[2][0:1, 1:W + 1], dt_[1][126:127, 1:W + 1])

        nc.sync.dma_start(out[0:128, :], dt_[0][0:128, 1:W + 1])
        nc.sync.dma_start(out[128:254, :], dt_[1][2:128, 1:W + 1])
        nc.sync.dma_start(out[254:352, :], dt_[2][2:100, 1:W + 1])

    nc.compile()
    return nc


def _band_matrix():
    a = np.zeros((128, 128), dtype=np.float32)
    idx = np.arange(128)
    a[idx, idx] = 1.0
    a[idx[:-1], idx[:-1] + 1] = 1.0
    a[idx[1:], idx[1:] - 1] = 1.0
    return a


_NC_CACHE = None


def kernel(guidance: np.ndarray, blur_depth: np.ndarray) -> np.ndarray:
    """Full inputs in, full output out. Shards batch across 8 NeuronCores."""
    global _NC_CACHE
    assert guidance.shape == (B, C, H, W), guidance.shape
    assert blur_depth.shape == (B, 1, H, W), blur_depth.shape
    if _NC_CACHE is None:
        _NC_CACHE = _build_nc()
    nc = _NC_CACHE
    band = _band_matrix()
    in_maps = [
        {
            "g": np.ascontiguousarray(guidance[b], dtype=np.float32),
            "d": np.ascontiguousarray(blur_depth[b, 0], dtype=np.float32),
            "band": band,
        }
        for b in range(B)
    ]
    res = run_bass_kernel_spmd(nc, in_maps, core_ids=list(range(N_CORES)))
    out = np.stack([res.results[b]["out"] for b in range(B)])[:, None]
    return out.astype(np.float32)
